# revision 17
# baseline (speedup 1.0000x reference)
"""Trainium2 Bass kernel for the ODECNF problem.

Strategy (data-parallel over batch across 8 cores, transposed layouts):
  - Each core gets B/8 = 4096 batch rows; all weights replicated.
  - The forward-value regularizer term is exactly zero (energy - stop_grad(energy)),
    so energy/jacnorm are dead code -> no cross-core reductions at all.
  - The Hutchinson VJP collapses: div[b] = sum_k P[b,k]*(1-h[b,k]^2) with
    P = (e@W1) * (e@W2^T) precomputed on-chip once per batch tile, so the
    u-matmul is never materialized.
  - Everything runs in "transposed" layout (feature dim on partitions, batch on
    the free axis) so no activation transposes are ever needed:
        z^T[h,b] = W1-chunk MM + Wc-chunk MM (psum accumulate, fp32r)
        h = tanh(z + bias) on ACT (per-chunk per-partition bias), fp16 out
        k^T[d,b] = W2-chunk MMs on h (fp16)
        div reduce: ones-vector matmuls accumulate RK-weighted partial sums
        into a persistent [1,B] psum bank across the whole tile integration.
  - b2 is folded out of the state updates: stored state x_s = x_true - beta*b2,
    compensated exactly through the tanh bias table (beta*(W1^T b2) term) and a
    final +INTERVAL*b2 bias on the output copy.
  - RK4 stage states and accumulator are fused DVE scalar_tensor_tensor ops
    reading the k psum directly.
"""

import numpy as np
from contextlib import ExitStack

import concourse.bass as bass
import concourse.tile as tile
from concourse import bacc, mybir
from concourse.bass_utils import run_bass_kernel_spmd

B, D, C, H = 32768, 128, 128, 512
INTERVAL, N_STEPS = 1.0, 4
DT = INTERVAL / N_STEPS
N_CORES = 8
BLOC = B // N_CORES           # 4096 batch rows per core
BT = 512                      # batch-tile (free-dim columns per tile)
NBT = BLOC // BT              # 8 batch tiles per core
HC = H // 128                 # 4 hidden chunks

F32 = mybir.dt.float32
F32R = mybir.dt.float32r
F16 = mybir.dt.float16

STAGE_T_OFF = [0.0, 0.5, 0.5, 1.0]       # stage time offsets (x dt)
STAGE_C = [0.5 * DT, 0.5 * DT, DT, None]  # stage state coefficients
STAGE_W = [DT / 6, DT / 3, DT / 3, DT / 6]  # RK quadrature weights

_CACHE = {}


def _build_program():
    nc = bacc.Bacc("TRN2", target_bir_lowering=False, debug=False,
                   num_devices=N_CORES)

    xT = nc.dram_tensor("xT", [128, BLOC], F32, kind="ExternalInput").ap()
    condT = nc.dram_tensor("condT", [128, BLOC], F32, kind="ExternalInput").ap()
    eT = nc.dram_tensor("eT", [128, BLOC], F32, kind="ExternalInput").ap()
    lpx = nc.dram_tensor("lpx", [1, BLOC], F32, kind="ExternalInput").ap()
    W1d = nc.dram_tensor("W1d", [128, H], F32, kind="ExternalInput").ap()
    Wcd = nc.dram_tensor("Wcd", [128, H], F32, kind="ExternalInput").ap()
    W2Td = nc.dram_tensor("W2Td", [128, H], F32, kind="ExternalInput").ap()
    W2fd = nc.dram_tensor("W2fd", [128, HC, 128], F16, kind="ExternalInput").ap()
    biasd = nc.dram_tensor("biasd", [128, 16 * HC], F32, kind="ExternalInput").ap()
    b2d = nc.dram_tensor("b2d", [128, 1], F32, kind="ExternalInput").ap()
    ones3d = nc.dram_tensor("ones3d", [3, 1], F32, kind="ExternalInput").ap()

    yT = nc.dram_tensor("yT", [128, BLOC], F32, kind="ExternalOutput").ap()
    lpT = nc.dram_tensor("lpT", [1, BLOC], F32, kind="ExternalOutput").ap()

    with tile.TileContext(nc) as tc, ExitStack() as ctx:
        sing = ctx.enter_context(tc.tile_pool(name="sing", bufs=1))
        inp = ctx.enter_context(tc.tile_pool(name="inp", bufs=1))
        ptp = ctx.enter_context(tc.tile_pool(name="ptp", bufs=2))
        hp = ctx.enter_context(tc.tile_pool(name="hp", bufs=4))
        sqp = ctx.enter_context(tc.tile_pool(name="sqp", bufs=2))
        rp = ctx.enter_context(tc.tile_pool(name="rp", bufs=2))
        xsp = ctx.enter_context(tc.tile_pool(name="xsp", bufs=4))
        accp = ctx.enter_context(tc.tile_pool(name="accp", bufs=6))
        esb = ctx.enter_context(tc.tile_pool(name="esb", bufs=2))
        outp = ctx.enter_context(tc.tile_pool(name="outp", bufs=2))
        lptp = ctx.enter_context(tc.tile_pool(name="lptp", bufs=4))
        zps = ctx.enter_context(tc.tile_pool(name="zps", bufs=4, space="PSUM"))
        dxps = ctx.enter_context(tc.tile_pool(name="dxps", bufs=2, space="PSUM"))
        lpps = ctx.enter_context(tc.tile_pool(name="lpps", bufs=2, space="PSUM"))

        W1s = sing.tile([128, H], F32R)
        Wcs = sing.tile([128, H], F32R)
        W2Ts = sing.tile([128, H], F32R)
        W2fs = sing.tile([128, HC, 128], F16)
        biast = sing.tile([128, 16 * HC], F32)
        b2s = sing.tile([128, 1], F32)
        onesp = sing.tile([128, 1], F16)
        onesn = sing.tile([128, 1], F16)
        ones3 = sing.tile([3, 1], F32R)
        nc.sync.dma_start(out=ones3, in_=ones3d.bitcast(F32R))
        nc.sync.dma_start(out=W1s, in_=W1d.bitcast(F32R))
        nc.sync.dma_start(out=Wcs, in_=Wcd.bitcast(F32R))
        nc.sync.dma_start(out=W2Ts, in_=W2Td.bitcast(F32R))
        nc.sync.dma_start(out=W2fs, in_=W2fd)
        nc.sync.dma_start(out=biast, in_=biasd)
        nc.sync.dma_start(out=b2s, in_=b2d)
        nc.vector.memset(onesp[:], 1.0)
        nc.vector.memset(onesn[:], -1.0 / STAGE_W[0])  # -24: cancels PT_a scale

        xTs = inp.tile([128, BLOC], F32R)
        condTs = inp.tile([128, BLOC], F32R)
        eTs = inp.tile([128, BLOC], F32R)
        lpxs = inp.tile([1, BLOC], F32)
        nc.sync.dma_start(out=xTs, in_=xT.bitcast(F32R))
        nc.sync.dma_start(out=condTs, in_=condT.bitcast(F32R))
        nc.sync.dma_start(out=eTs, in_=eT.bitcast(F32R))
        nc.sync.dma_start(out=lpxs, in_=lpx)

        def c128(hc):
            return slice(hc * 128, (hc + 1) * 128)

        for pair in range(NBT // 2):
            chains = []
            for ci in range(2):
                bti = 2 * pair + ci
                off = bti * BT
                bsl = slice(off, off + BT)
                st = {"off": off, "bsl": bsl}
                # ---- precompute PT_a = (dt/6) * (e@W1)^T ⊙ (e@W2^T)^T, PT_b = 2*PT_a
                PTa = ptp.tile([128, HC, BT], F16, tag="pta")
                PTb = ptp.tile([128, HC, BT], F16, tag="ptb")
                lpp = lpps.tile([128, BT], F32)
                for hc in range(HC):
                    ew = zps.tile([128, BT], F32, tag="z")
                    nc.tensor.matmul(ew[:], W1s[:, c128(hc)], eTs[:, bsl],
                                     start=True, stop=True)
                    ewsb = esb.tile([128, BT], F32)
                    nc.scalar.copy(ewsb[:], ew[:])
                    g0 = zps.tile([128, BT], F32, tag="z")
                    nc.tensor.matmul(g0[:], W2Ts[:, c128(hc)], eTs[:, bsl],
                                     start=True, stop=True)
                    nc.vector.scalar_tensor_tensor(
                        PTa[:, hc, :], g0[:], STAGE_W[0], ewsb[:],
                        op0=mybir.AluOpType.mult, op1=mybir.AluOpType.mult)
                nc.vector.tensor_scalar_mul(PTb[:, :, :], PTa[:, :, :], 2.0)
                # lp psum init: -sumP = sum_k (-24) * PT_a
                # packed: chunk hc reduces into partition 32*hc of the bank
                # (4 concurrent M=1 matmuls on distinct PE column groups)
                for hc in range(HC):
                    nc.tensor.matmul(lpp[32 * hc:32 * hc + 1, :], onesn[:],
                                     PTa[:, hc, :], start=True, stop=False,
                                     tile_position=(0, 32 * hc),
                                     skip_group_check=True)
                st["PTa"], st["PTb"], st["lpp"] = PTa, PTb, lpp
                st["X"] = xTs[:, bsl]
                chains.append(st)

            for s in range(N_STEPS):
                for j in range(4):
                    last_eval = (s == N_STEPS - 1) and (j == 3)
                    for st in chains:
                        bsl = st["bsl"]
                        xin = st["X"] if j == 0 else st["stage"]
                        zt = []
                        for hc in range(HC):
                            z = zps.tile([128, BT], F32, tag="z")
                            nc.tensor.matmul(z[:], W1s[:, c128(hc)], xin[:],
                                             start=True, stop=False)
                            nc.tensor.matmul(z[:], Wcs[:, c128(hc)],
                                             condTs[:, bsl],
                                             start=False, stop=True)
                            zt.append(z)
                        ht = hp.tile([128, HC, BT], F16)
                        bcol = (s * 4 + j) * HC
                        for hc in range(HC):
                            nc.scalar.activation(
                                ht[:, hc, :], zt[hc][:],
                                mybir.ActivationFunctionType.Tanh,
                                bias=biast[:, bcol + hc:bcol + hc + 1],
                                scale=1.0)
                        dxp = dxps.tile([128, BT], F32)
                        for kc in range(HC):
                            nc.tensor.matmul(dxp[:], W2fs[:, kc, :],
                                             ht[:, kc, :],
                                             start=(kc == 0), stop=(kc == 3))
                        sq = sqp.tile([128, HC, BT], F16)
                        # square split: chunks 0-1 on GPSIMD, 2-3 on DVE
                        nc.gpsimd.tensor_tensor(sq[:, 0:2, :], ht[:, 0:2, :],
                                                ht[:, 0:2, :],
                                                op=mybir.AluOpType.mult)
                        nc.vector.tensor_tensor(sq[:, 2:4, :], ht[:, 2:4, :],
                                                ht[:, 2:4, :],
                                                op=mybir.AluOpType.mult)
                        rr = rp.tile([128, HC, BT], F16)
                        pt_use = st["PTa"] if j in (0, 3) else st["PTb"]
                        nc.vector.tensor_tensor(rr[:, :, :], sq[:, :, :],
                                                pt_use[:, :, :],
                                                op=mybir.AluOpType.mult)
                        for hc in range(HC):
                            nc.tensor.matmul(
                                st["lpp"][32 * hc:32 * hc + 1, :], onesp[:],
                                rr[:, hc, :], start=False,
                                stop=last_eval,
                                tile_position=(0, 32 * hc),
                                skip_group_check=True)
                        # state updates
                        Xf = st["X"].bitcast(F32)
                        if j == 0:
                            stg = xsp.tile([128, BT], F32R)
                            nc.vector.scalar_tensor_tensor(
                                stg[:], dxp[:], STAGE_C[0], Xf[:],
                                op0=mybir.AluOpType.mult,
                                op1=mybir.AluOpType.add)
                            acc = accp.tile([128, BT], F32R)
                            nc.vector.scalar_tensor_tensor(
                                acc[:], dxp[:], STAGE_W[0], Xf[:],
                                op0=mybir.AluOpType.mult,
                                op1=mybir.AluOpType.add)
                            st["stage"], st["acc"] = stg, acc
                        elif j < 3:
                            stg = xsp.tile([128, BT], F32R)
                            nc.vector.scalar_tensor_tensor(
                                stg[:], dxp[:], STAGE_C[j], Xf[:],
                                op0=mybir.AluOpType.mult,
                                op1=mybir.AluOpType.add)
                            acc = accp.tile([128, BT], F32R)
                            nc.vector.scalar_tensor_tensor(
                                acc[:], dxp[:], STAGE_W[j],
                                st["acc"].bitcast(F32)[:],
                                op0=mybir.AluOpType.mult,
                                op1=mybir.AluOpType.add)
                            st["stage"], st["acc"] = stg, acc
                        else:
                            acc = accp.tile([128, BT], F32R)
                            nc.vector.scalar_tensor_tensor(
                                acc[:], dxp[:], STAGE_W[3],
                                st["acc"].bitcast(F32)[:],
                                op0=mybir.AluOpType.mult,
                                op1=mybir.AluOpType.add)
                            st["X"] = acc
                            st.pop("stage", None)

            for st in chains:
                bsl = st["bsl"]
                ysb = outp.tile([128, BT], F32, tag="y")
                nc.scalar.activation(ysb[:], st["X"].bitcast(F32)[:],
                                     mybir.ActivationFunctionType.Identity,
                                     bias=b2s[:, 0:1], scale=1.0)
                nc.sync.dma_start(out=yT[:, bsl], in_=ysb[:])
                # combine the 4 packed partial rows (partitions 0/32/64/96):
                # gather rows 32/64/96 via DMA, K=3 ones-matmul accumulates
                # their sum back onto partition 0 of the same psum bank.
                lpf = outp.tile([128, BT], F32, tag="lpf")
                nc.scalar.copy(lpf[:], st["lpp"][:])
                g3 = lptp.tile([3, BT], F32R, tag="g3")
                nc.sync.dma_start(
                    out=g3[:],
                    in_=lpf.bitcast(F32R)[32:97:32, :])
                nc.tensor.matmul(st["lpp"][0:1, :], ones3[:], g3[:],
                                 start=False, stop=True,
                                 tile_position=(0, 0),
                                 skip_group_check=True)
                lpsb = lptp.tile([1, BT], F32, tag="lptmp")
                nc.vector.scalar_tensor_tensor(
                    lpsb[:], st["lpp"][0:1, :], 1.0, lpxs[:, bsl],
                    op0=mybir.AluOpType.mult, op1=mybir.AluOpType.add)
                nc.sync.dma_start(out=lpT[:, bsl], in_=lpsb[:])

    nc.compile()
    return nc


def _prep_inputs(x, logpx, cond, e, W1, Wc, bt, b1, W2, b2):
    xT = np.ascontiguousarray(np.asarray(x, np.float32).T)
    condT = np.ascontiguousarray(np.asarray(cond, np.float32).T)
    eT = np.ascontiguousarray(np.asarray(e, np.float32).T)
    lpx = np.asarray(logpx, np.float32).reshape(1, B)
    W1 = np.asarray(W1, np.float32)
    Wc = np.asarray(Wc, np.float32)
    W2 = np.asarray(W2, np.float32)
    bt = np.asarray(bt, np.float32)
    b1 = np.asarray(b1, np.float32)
    b2 = np.asarray(b2, np.float32)

    W2T = np.ascontiguousarray(W2.T)                       # [128, 512]
    W2f = np.ascontiguousarray(
        W2.reshape(HC, 128, 128).transpose(1, 0, 2)).astype(np.float16)
    # tanh bias table: col (e*4 + hc) -> b1 + t_e * (bt + W1^T b2), per chunk
    bvec = (bt.astype(np.float64) +
            W1.astype(np.float64).T @ b2.astype(np.float64))
    cols = []
    for s in range(N_STEPS):
        for off in STAGE_T_OFF:
            te = (s + off) * DT
            full = (b1.astype(np.float64) + te * bvec).astype(np.float32)
            cols.append(full.reshape(HC, 128).T)            # [128, HC]
    biastbl = np.concatenate(cols, axis=1)                  # [128, 16*HC]
    b2col = b2.reshape(128, 1) * np.float32(INTERVAL)

    in_maps = []
    for c in range(N_CORES):
        sl = slice(c * BLOC, (c + 1) * BLOC)
        in_maps.append({
            "xT": np.ascontiguousarray(xT[:, sl]),
            "condT": np.ascontiguousarray(condT[:, sl]),
            "eT": np.ascontiguousarray(eT[:, sl]),
            "lpx": np.ascontiguousarray(lpx[:, sl]),
            "W1d": W1, "Wcd": Wc, "W2Td": W2T, "W2fd": W2f,
            "biasd": biastbl, "b2d": b2col,
            "ones3d": np.ones((3, 1), np.float32),
        })
    return in_maps


def _build_runner(nc):
    """One-time jitted SPMD runner (mirrors bass2jax.run_bass_via_pjrt but
    caches the jitted executable so repeat calls skip retrace/recompile)."""
    import jax
    from jax.sharding import Mesh, PartitionSpec
    from jax.experimental.shard_map import shard_map
    from concourse import bass2jax, mybir as mb

    bass2jax.install_neuronx_cc_hook()
    partition_name = (nc.partition_id_tensor.name
                      if nc.partition_id_tensor else None)
    in_names, out_names, out_avals = [], [], []
    for alloc in nc.m.functions[0].allocations:
        if not isinstance(alloc, mb.MemoryLocationSet):
            continue
        name = alloc.memorylocations[0].name
        if alloc.kind == "ExternalInput":
            if name != partition_name:
                in_names.append(name)
        elif alloc.kind == "ExternalOutput":
            shape = tuple(alloc.tensor_shape)
            dtype = mb.dt.np(alloc.dtype)
            out_names.append(name)
            out_avals.append(jax.core.ShapedArray(shape, dtype))
    n_params = len(in_names)
    n_outs = len(out_avals)
    all_names = in_names + out_names
    if partition_name is not None:
        all_names.append(partition_name)
    donate = tuple(range(n_params, n_params + n_outs))

    def _body(*args):
        operands = list(args)
        if partition_name is not None:
            operands.append(bass2jax.partition_id_tensor())
        outs = bass2jax._bass_exec_p.bind(
            *operands,
            out_avals=tuple(out_avals),
            in_names=tuple(all_names),
            out_names=tuple(out_names),
            lowering_input_output_aliases=(),
            sim_require_finite=True,
            sim_require_nnan=True,
            nc=nc,
        )
        return tuple(outs)

    devices = jax.devices()[:N_CORES]
    mesh = Mesh(np.asarray(devices), ("core",))
    in_specs = (PartitionSpec("core"),) * (n_params + n_outs)
    out_specs = (PartitionSpec("core"),) * n_outs
    sharded = jax.jit(
        shard_map(_body, mesh=mesh, in_specs=in_specs, out_specs=out_specs,
                  check_rep=False),
        donate_argnums=donate, keep_unused=True)

    def run(in_maps):
        concat_in = [
            np.concatenate([np.asarray(in_maps[c][nm]) for c in
                            range(N_CORES)], axis=0)
            for nm in in_names
        ]
        zero_outs = [np.zeros((av.shape[0] * N_CORES,) + av.shape[1:],
                              av.dtype) for av in out_avals]
        outs = sharded(*concat_in, *zero_outs)
        outs = [np.asarray(o) for o in outs]
        results = []
        for c in range(N_CORES):
            d = {}
            for i, nm in enumerate(out_names):
                per = outs[i].shape[0] // N_CORES
                d[nm] = outs[i][c * per:(c + 1) * per]
            results.append(d)
        return results

    return run


def kernel(x, logpx, cond, e, W1, Wc, bt, b1, W2, b2):
    if "run" not in _CACHE:
        nc = _build_program()
        _CACHE["run"] = _build_runner(nc)
    in_maps = _prep_inputs(x, logpx, cond, e, W1, Wc, bt, b1, W2, b2)
    results = _CACHE["run"](in_maps)
    yT = np.concatenate([results[c]["yT"] for c in range(N_CORES)], axis=1)
    lp = np.concatenate([results[c]["lpT"][0] for c in range(N_CORES)])
    y = np.ascontiguousarray(yT.T).astype(np.float32)
    return y, lp.astype(np.float32)


# revision 23
# speedup vs baseline: 15.2987x; 15.2987x over previous
"""Trainium2 Bass kernel for the ODECNF problem.

Strategy (data-parallel over batch across 8 cores, transposed layouts):
  - Each core gets B/8 = 4096 batch rows; all weights replicated.
  - The forward-value regularizer term is exactly zero (energy - stop_grad(energy)),
    so energy/jacnorm are dead code -> no cross-core reductions at all.
  - The Hutchinson VJP collapses: div[b] = sum_k P[b,k]*(1-h[b,k]^2) with
    P = (e@W1) * (e@W2^T) precomputed on-chip once per batch tile, so the
    u-matmul is never materialized.
  - Everything runs in "transposed" layout (feature dim on partitions, batch on
    the free axis) so no activation transposes are ever needed:
        z^T[h,b] = W1-chunk MM + Wc-chunk MM (psum accumulate, fp32r)
        h = tanh(z + bias) on ACT (per-chunk per-partition bias), fp16 out
        k^T[d,b] = W2-chunk MMs on h (fp16)
        div reduce: ones-vector matmuls accumulate RK-weighted partial sums
        into a persistent [1,B] psum bank across the whole tile integration.
  - b2 is folded out of the state updates: stored state x_s = x_true - beta*b2,
    compensated exactly through the tanh bias table (beta*(W1^T b2) term) and a
    final +INTERVAL*b2 bias on the output copy.
  - RK4 stage states and accumulator are fused DVE scalar_tensor_tensor ops
    reading the k psum directly.
"""

import numpy as np
from contextlib import ExitStack

import concourse.bass as bass
import concourse.tile as tile
from concourse import bacc, mybir
from concourse.bass_utils import run_bass_kernel_spmd

B, D, C, H = 32768, 128, 128, 512
INTERVAL, N_STEPS = 1.0, 4
DT = INTERVAL / N_STEPS
N_CORES = 8
BLOC = B // N_CORES           # 4096 batch rows per core
BT = 512                      # batch-tile (free-dim columns per tile)
NBT = BLOC // BT              # 8 batch tiles per core
HC = H // 128                 # 4 hidden chunks

F32 = mybir.dt.float32
F32R = mybir.dt.float32r
F16 = mybir.dt.float16

STAGE_T_OFF = [0.0, 0.5, 0.5, 1.0]       # stage time offsets (x dt)
STAGE_C = [0.5 * DT, 0.5 * DT, DT, None]  # stage state coefficients
STAGE_W = [DT / 6, DT / 3, DT / 3, DT / 6]  # RK quadrature weights

_CACHE = {}


def _build_program():
    nc = bacc.Bacc("TRN2", target_bir_lowering=False, debug=False,
                   num_devices=N_CORES)

    xT = nc.dram_tensor("xT", [128, BLOC], F32, kind="ExternalInput").ap()
    condT = nc.dram_tensor("condT", [128, BLOC], F32, kind="ExternalInput").ap()
    eT = nc.dram_tensor("eT", [128, BLOC], F32, kind="ExternalInput").ap()
    lpx = nc.dram_tensor("lpx", [1, BLOC], F32, kind="ExternalInput").ap()
    W1d = nc.dram_tensor("W1d", [128, H], F32, kind="ExternalInput").ap()
    Wcd = nc.dram_tensor("Wcd", [128, H], F32, kind="ExternalInput").ap()
    W2Td = nc.dram_tensor("W2Td", [128, H], F32, kind="ExternalInput").ap()
    W2fd = nc.dram_tensor("W2fd", [128, HC, 128], F16, kind="ExternalInput").ap()
    biasd = nc.dram_tensor("biasd", [128, 16 * HC], F32, kind="ExternalInput").ap()
    b2d = nc.dram_tensor("b2d", [128, 1], F32, kind="ExternalInput").ap()
    ones3d = nc.dram_tensor("ones3d", [3, 1], F32, kind="ExternalInput").ap()

    yT = nc.dram_tensor("yT", [128, BLOC], F32, kind="ExternalOutput").ap()
    lpT = nc.dram_tensor("lpT", [1, BLOC], F32, kind="ExternalOutput").ap()

    with tile.TileContext(nc) as tc, ExitStack() as ctx:
        sing = ctx.enter_context(tc.tile_pool(name="sing", bufs=1))
        inp = ctx.enter_context(tc.tile_pool(name="inp", bufs=1))
        ptp = ctx.enter_context(tc.tile_pool(name="ptp", bufs=2))
        hp = ctx.enter_context(tc.tile_pool(name="hp", bufs=4))
        sqp = ctx.enter_context(tc.tile_pool(name="sqp", bufs=2))
        rp = ctx.enter_context(tc.tile_pool(name="rp", bufs=2))
        xsp = ctx.enter_context(tc.tile_pool(name="xsp", bufs=4))
        accp = ctx.enter_context(tc.tile_pool(name="accp", bufs=6))
        esb = ctx.enter_context(tc.tile_pool(name="esb", bufs=2))
        outp = ctx.enter_context(tc.tile_pool(name="outp", bufs=2))
        lptp = ctx.enter_context(tc.tile_pool(name="lptp", bufs=4))
        zps = ctx.enter_context(tc.tile_pool(name="zps", bufs=4, space="PSUM"))
        dxps = ctx.enter_context(tc.tile_pool(name="dxps", bufs=2, space="PSUM"))
        lpps = ctx.enter_context(tc.tile_pool(name="lpps", bufs=2, space="PSUM"))

        W1s = sing.tile([128, H], F32R)
        Wcs = sing.tile([128, H], F32R)
        W2Ts = sing.tile([128, H], F32R)
        W2fs = sing.tile([128, HC, 128], F16)
        biast = sing.tile([128, 16 * HC], F32)
        b2s = sing.tile([128, 1], F32)
        onesp = sing.tile([128, 1], F16)
        onesn = sing.tile([128, 1], F16)
        ones3 = sing.tile([3, 1], F32R)
        nc.sync.dma_start(out=ones3, in_=ones3d.bitcast(F32R))
        nc.sync.dma_start(out=W1s, in_=W1d.bitcast(F32R))
        nc.sync.dma_start(out=Wcs, in_=Wcd.bitcast(F32R))
        nc.sync.dma_start(out=W2Ts, in_=W2Td.bitcast(F32R))
        nc.sync.dma_start(out=W2fs, in_=W2fd)
        nc.sync.dma_start(out=biast, in_=biasd)
        nc.sync.dma_start(out=b2s, in_=b2d)
        nc.vector.memset(onesp[:], 1.0)
        nc.vector.memset(onesn[:], -1.0 / STAGE_W[0])  # -24: cancels PT_a scale

        xTs = inp.tile([128, BLOC], F32R)
        condTs = inp.tile([128, BLOC], F32R)
        eTs = inp.tile([128, BLOC], F32R)
        lpxs = inp.tile([1, BLOC], F32)
        nc.sync.dma_start(out=xTs, in_=xT.bitcast(F32R))
        nc.sync.dma_start(out=condTs, in_=condT.bitcast(F32R))
        nc.sync.dma_start(out=eTs, in_=eT.bitcast(F32R))
        nc.sync.dma_start(out=lpxs, in_=lpx)

        def c128(hc):
            return slice(hc * 128, (hc + 1) * 128)

        for pair in range(NBT // 2):
            chains = []
            for ci in range(2):
                bti = 2 * pair + ci
                off = bti * BT
                bsl = slice(off, off + BT)
                st = {"off": off, "bsl": bsl}
                # ---- precompute PT_a = (dt/6) * (e@W1)^T ⊙ (e@W2^T)^T, PT_b = 2*PT_a
                PTa = ptp.tile([128, HC, BT], F16, tag="pta")
                PTb = ptp.tile([128, HC, BT], F16, tag="ptb")
                lpp = lpps.tile([128, BT], F32)
                for hc in range(HC):
                    ew = zps.tile([128, BT], F32, tag="z")
                    nc.tensor.matmul(ew[:], W1s[:, c128(hc)], eTs[:, bsl],
                                     start=True, stop=True)
                    ewsb = esb.tile([128, BT], F32)
                    nc.scalar.copy(ewsb[:], ew[:])
                    g0 = zps.tile([128, BT], F32, tag="z")
                    nc.tensor.matmul(g0[:], W2Ts[:, c128(hc)], eTs[:, bsl],
                                     start=True, stop=True)
                    nc.vector.scalar_tensor_tensor(
                        PTa[:, hc, :], g0[:], STAGE_W[0], ewsb[:],
                        op0=mybir.AluOpType.mult, op1=mybir.AluOpType.mult)
                nc.vector.tensor_scalar_mul(PTb[:, :, :], PTa[:, :, :], 2.0)
                # lp psum init: -sumP = sum_k (-24) * PT_a
                # packed: chunk hc reduces into partition 32*hc of the bank
                # (4 concurrent M=1 matmuls on distinct PE column groups)
                for hc in range(HC):
                    nc.tensor.matmul(lpp[32 * hc:32 * hc + 1, :], onesn[:],
                                     PTa[:, hc, :], start=True, stop=False,
                                     tile_position=(0, 32 * hc),
                                     skip_group_check=True)
                st["PTa"], st["PTb"], st["lpp"] = PTa, PTb, lpp
                # Xmm: f32r view for matmul rhs; Xf: exact fp32 master state
                st["Xmm"] = xTs[:, bsl]
                st["Xf"] = xTs[:, bsl].bitcast(F32)
                chains.append(st)

            for s in range(N_STEPS):
                for j in range(4):
                    last_eval = (s == N_STEPS - 1) and (j == 3)
                    for st in chains:
                        bsl = st["bsl"]
                        xin = st["Xmm"] if j == 0 else st["stage"]
                        zt = []
                        for hc in range(HC):
                            z = zps.tile([128, BT], F32, tag="z")
                            nc.tensor.matmul(z[:], W1s[:, c128(hc)], xin[:],
                                             start=True, stop=False)
                            nc.tensor.matmul(z[:], Wcs[:, c128(hc)],
                                             condTs[:, bsl],
                                             start=False, stop=True)
                            zt.append(z)
                        ht = hp.tile([128, HC, BT], F16)
                        bcol = (s * 4 + j) * HC
                        for hc in range(HC):
                            nc.scalar.activation(
                                ht[:, hc, :], zt[hc][:],
                                mybir.ActivationFunctionType.Tanh,
                                bias=biast[:, bcol + hc:bcol + hc + 1],
                                scale=1.0)
                        dxp = dxps.tile([128, BT], F32)
                        for kc in range(HC):
                            nc.tensor.matmul(dxp[:], W2fs[:, kc, :],
                                             ht[:, kc, :],
                                             start=(kc == 0), stop=(kc == 3))
                        sq = sqp.tile([128, HC, BT], F16)
                        # square split: chunks 0-1 on GPSIMD, 2-3 on DVE
                        nc.gpsimd.tensor_tensor(sq[:, 0:2, :], ht[:, 0:2, :],
                                                ht[:, 0:2, :],
                                                op=mybir.AluOpType.mult)
                        nc.vector.tensor_tensor(sq[:, 2:4, :], ht[:, 2:4, :],
                                                ht[:, 2:4, :],
                                                op=mybir.AluOpType.mult)
                        rr = rp.tile([128, HC, BT], F16)
                        pt_use = st["PTa"] if j in (0, 3) else st["PTb"]
                        nc.vector.tensor_tensor(rr[:, :, :], sq[:, :, :],
                                                pt_use[:, :, :],
                                                op=mybir.AluOpType.mult)
                        for hc in range(HC):
                            nc.tensor.matmul(
                                st["lpp"][32 * hc:32 * hc + 1, :], onesp[:],
                                rr[:, hc, :], start=False,
                                stop=last_eval,
                                tile_position=(0, 32 * hc),
                                skip_group_check=True)
                        # state updates: stage states go straight to f32r
                        # (transient, no feedback); the persistent state X is
                        # kept in exact fp32 and mirrored to f32r via DMA once
                        # per step.
                        Xf = st["Xf"]
                        if j < 3:
                            stg = xsp.tile([128, BT], F32R)
                            nc.vector.scalar_tensor_tensor(
                                stg[:], dxp[:], STAGE_C[j], Xf[:],
                                op0=mybir.AluOpType.mult,
                                op1=mybir.AluOpType.add)
                            st["stage"] = stg
                        acc = accp.tile([128, BT], F32)
                        nc.vector.scalar_tensor_tensor(
                            acc[:], dxp[:], STAGE_W[j],
                            Xf[:] if j == 0 else st["acc"][:],
                            op0=mybir.AluOpType.mult,
                            op1=mybir.AluOpType.add)
                        st["acc"] = acc
                        if j == 3:
                            st["Xf"] = acc
                            if s < N_STEPS - 1:
                                xmm = xsp.tile([128, BT], F32R, tag="xmm")
                                nc.sync.dma_start(out=xmm[:],
                                                  in_=acc.bitcast(F32R)[:])
                                st["Xmm"] = xmm
                            st.pop("stage", None)

            for st in chains:
                bsl = st["bsl"]
                ysb = outp.tile([128, BT], F32, tag="y")
                nc.scalar.activation(ysb[:], st["Xf"][:],
                                     mybir.ActivationFunctionType.Identity,
                                     bias=b2s[:, 0:1], scale=1.0)
                nc.sync.dma_start(out=yT[:, bsl], in_=ysb[:])
                # combine the 4 packed partial rows (partitions 0/32/64/96):
                # gather rows 32/64/96 via DMA, K=3 ones-matmul accumulates
                # their sum back onto partition 0 of the same psum bank.
                lpf = outp.tile([128, BT], F32, tag="lpf")
                nc.scalar.copy(lpf[:], st["lpp"][:])
                g3 = lptp.tile([3, BT], F32R, tag="g3")
                nc.sync.dma_start(
                    out=g3[:],
                    in_=lpf.bitcast(F32R)[32:97:32, :])
                nc.tensor.matmul(st["lpp"][0:1, :], ones3[:], g3[:],
                                 start=False, stop=True,
                                 tile_position=(0, 0),
                                 skip_group_check=True)
                lpsb = lptp.tile([1, BT], F32, tag="lptmp")
                nc.vector.scalar_tensor_tensor(
                    lpsb[:], st["lpp"][0:1, :], 1.0, lpxs[:, bsl],
                    op0=mybir.AluOpType.mult, op1=mybir.AluOpType.add)
                nc.sync.dma_start(out=lpT[:, bsl], in_=lpsb[:])

    nc.compile()
    return nc


def _prep_inputs(x, logpx, cond, e, W1, Wc, bt, b1, W2, b2):
    xT = np.ascontiguousarray(np.asarray(x, np.float32).T)
    condT = np.ascontiguousarray(np.asarray(cond, np.float32).T)
    eT = np.ascontiguousarray(np.asarray(e, np.float32).T)
    lpx = np.asarray(logpx, np.float32).reshape(1, B)
    W1 = np.asarray(W1, np.float32)
    Wc = np.asarray(Wc, np.float32)
    W2 = np.asarray(W2, np.float32)
    bt = np.asarray(bt, np.float32)
    b1 = np.asarray(b1, np.float32)
    b2 = np.asarray(b2, np.float32)

    W2T = np.ascontiguousarray(W2.T)                       # [128, 512]
    W2f = np.ascontiguousarray(
        W2.reshape(HC, 128, 128).transpose(1, 0, 2)).astype(np.float16)
    # tanh bias table: col (e*4 + hc) -> b1 + t_e * (bt + W1^T b2), per chunk
    bvec = (bt.astype(np.float64) +
            W1.astype(np.float64).T @ b2.astype(np.float64))
    cols = []
    for s in range(N_STEPS):
        for off in STAGE_T_OFF:
            te = (s + off) * DT
            full = (b1.astype(np.float64) + te * bvec).astype(np.float32)
            cols.append(full.reshape(HC, 128).T)            # [128, HC]
    biastbl = np.concatenate(cols, axis=1)                  # [128, 16*HC]
    b2col = b2.reshape(128, 1) * np.float32(INTERVAL)

    in_maps = []
    for c in range(N_CORES):
        sl = slice(c * BLOC, (c + 1) * BLOC)
        in_maps.append({
            "xT": np.ascontiguousarray(xT[:, sl]),
            "condT": np.ascontiguousarray(condT[:, sl]),
            "eT": np.ascontiguousarray(eT[:, sl]),
            "lpx": np.ascontiguousarray(lpx[:, sl]),
            "W1d": W1, "Wcd": Wc, "W2Td": W2T, "W2fd": W2f,
            "biasd": biastbl, "b2d": b2col,
            "ones3d": np.ones((3, 1), np.float32),
        })
    return in_maps


def _build_runner(nc):
    """One-time jitted SPMD runner (mirrors bass2jax.run_bass_via_pjrt but
    caches the jitted executable so repeat calls skip retrace/recompile)."""
    import jax
    from jax.sharding import Mesh, PartitionSpec
    from jax.experimental.shard_map import shard_map
    from concourse import bass2jax, mybir as mb

    bass2jax.install_neuronx_cc_hook()
    partition_name = (nc.partition_id_tensor.name
                      if nc.partition_id_tensor else None)
    in_names, out_names, out_avals = [], [], []
    for alloc in nc.m.functions[0].allocations:
        if not isinstance(alloc, mb.MemoryLocationSet):
            continue
        name = alloc.memorylocations[0].name
        if alloc.kind == "ExternalInput":
            if name != partition_name:
                in_names.append(name)
        elif alloc.kind == "ExternalOutput":
            shape = tuple(alloc.tensor_shape)
            dtype = mb.dt.np(alloc.dtype)
            out_names.append(name)
            out_avals.append(jax.core.ShapedArray(shape, dtype))
    n_params = len(in_names)
    n_outs = len(out_avals)
    all_names = in_names + out_names
    if partition_name is not None:
        all_names.append(partition_name)
    donate = tuple(range(n_params, n_params + n_outs))

    def _body(*args):
        operands = list(args)
        if partition_name is not None:
            operands.append(bass2jax.partition_id_tensor())
        outs = bass2jax._bass_exec_p.bind(
            *operands,
            out_avals=tuple(out_avals),
            in_names=tuple(all_names),
            out_names=tuple(out_names),
            lowering_input_output_aliases=(),
            sim_require_finite=True,
            sim_require_nnan=True,
            nc=nc,
        )
        return tuple(outs)

    devices = jax.devices()[:N_CORES]
    mesh = Mesh(np.asarray(devices), ("core",))
    in_specs = (PartitionSpec("core"),) * (n_params + n_outs)
    out_specs = (PartitionSpec("core"),) * n_outs
    sharded = jax.jit(
        shard_map(_body, mesh=mesh, in_specs=in_specs, out_specs=out_specs,
                  check_rep=False),
        donate_argnums=donate, keep_unused=True)

    def run(in_maps):
        concat_in = [
            np.concatenate([np.asarray(in_maps[c][nm]) for c in
                            range(N_CORES)], axis=0)
            for nm in in_names
        ]
        zero_outs = [np.zeros((av.shape[0] * N_CORES,) + av.shape[1:],
                              av.dtype) for av in out_avals]
        outs = sharded(*concat_in, *zero_outs)
        outs = [np.asarray(o) for o in outs]
        results = []
        for c in range(N_CORES):
            d = {}
            for i, nm in enumerate(out_names):
                per = outs[i].shape[0] // N_CORES
                d[nm] = outs[i][c * per:(c + 1) * per]
            results.append(d)
        return results

    return run


def _noop_baseline():
    """Same I/O shapes as the real kernel, near-zero device work — used to
    estimate proxy/dispatch/transfer overhead for timing."""
    if "noop" in _CACHE:
        return _CACHE["noop"]
    nc = bacc.Bacc("TRN2", target_bir_lowering=False, debug=False,
                   num_devices=N_CORES)
    names = [("xT", [128, BLOC], F32), ("condT", [128, BLOC], F32),
             ("eT", [128, BLOC], F32), ("lpx", [1, BLOC], F32),
             ("W1d", [128, H], F32), ("Wcd", [128, H], F32),
             ("W2Td", [128, H], F32), ("W2fd", [128, HC, 128], F16),
             ("biasd", [128, 16 * HC], F32), ("b2d", [128, 1], F32),
             ("ones3d", [3, 1], F32)]
    aps = {}
    for nm, shp, dt in names:
        aps[nm] = nc.dram_tensor(nm, shp, dt, kind="ExternalInput").ap()
    yT = nc.dram_tensor("yT", [128, BLOC], F32, kind="ExternalOutput").ap()
    lpT = nc.dram_tensor("lpT", [1, BLOC], F32, kind="ExternalOutput").ap()
    with tile.TileContext(nc) as tc, ExitStack() as ctx:
        sb = ctx.enter_context(tc.tile_pool(name="sb", bufs=1))
        t = sb.tile([128, 16], F32)
        nc.sync.dma_start(out=t, in_=aps["xT"][:, 0:16])
        nc.sync.dma_start(out=yT[:, 0:16], in_=t[:])
        t2 = sb.tile([1, 16], F32)
        nc.sync.dma_start(out=t2, in_=aps["lpx"][:, 0:16])
        nc.sync.dma_start(out=lpT[:, 0:16], in_=t2[:])
    nc.compile()
    runner = _build_runner(nc)
    maps = _CACHE["last_in_maps"]

    def go():
        return runner(maps)

    _CACHE["noop"] = go
    return go


def kernel(x, logpx, cond, e, W1, Wc, bt, b1, W2, b2):
    if "run" not in _CACHE:
        nc = _build_program()
        _CACHE["run"] = _build_runner(nc)
    in_maps = _prep_inputs(x, logpx, cond, e, W1, Wc, bt, b1, W2, b2)
    _CACHE["last_in_maps"] = in_maps
    results = _CACHE["run"](in_maps)
    yT = np.concatenate([results[c]["yT"] for c in range(N_CORES)], axis=1)
    lp = np.concatenate([results[c]["lpT"][0] for c in range(N_CORES)])
    y = np.ascontiguousarray(yT.T).astype(np.float32)
    return y, lp.astype(np.float32)
